# revision 1
# baseline (speedup 1.0000x reference)
"""BiLSTM-CRF loss kernel for Trainium2, data-parallel over batch on 8 NeuronCores.

Per-core program (B_local=16 sequences, S=512, T=20 tags, E=100, H=128):
  1. Embedding gather (indirect DMA) + PE transpose -> xsT [101, S*16] (ones row
     appended so LSTM input-projection matmuls fold in the bias).
  2. Input projections P = W_ih_aug @ xsT for both directions, all gates, written
     to internal DRAM gate-interleaved so the recurrence streams one slice per step.
  3. fwd+bwd LSTM recurrences interleaved (two independent dependency chains),
     transposed state layout hT/cT [128, 16]; 4 matmuls/step/dir into PSUM [128,64].
  4. Emissions em^T [20, S*16] = W_out @ [hf;hb] + b_out, plus exp(em) for the CRF DP.
  5. CRF forward DP in exp domain: A <- (exp(trans)^T @ A) * expE_t, renormalized by
     column sums (ones-matmul + K=1 broadcast matmul) every NORM_EVERY steps,
     log-partition accumulated in logZ.  Two independent batch half-chains.
  6. CRF numerator via one-hot of tags [20, S*16]: emit/trans/start/end scores with
     elementwise ops + trans^T@OH matmul + segmented reduces.
  7. Output per-core [1,16] = path_score - log_partition; host computes -mean.

mask is all ones for this problem (spec fill=ones), so masking is elided and
seq_ends = S-1.
"""

import os
import sys

import numpy as np

sys.path.insert(0, "/opt/trn_rl_repo")

import concourse.bass as bass
import concourse.mybir as mybir
import concourse.tile as tile
from concourse import bacc
from concourse.bass import IndirectOffsetOnAxis
from concourse.masks import make_identity

AF = mybir.ActivationFunctionType
ALU = mybir.AluOpType
AX = mybir.AxisListType
F32 = mybir.dt.float32
BF16 = mybir.dt.bfloat16
I32 = mybir.dt.int32

V, T, E, HD = 32000, 20, 100, 256
H = 128
B, S = 128, 512
NCORES = 8
BL = B // NCORES  # 16 sequences per core
NORM_EVERY = 4

# gate positions in the per-step PSUM [128, 64]: i,f,o contiguous for one fused
# sigmoid; g (cell) last for tanh.  Source column offset in the [*,4H] weight
# layouts (PyTorch gate order i,f,g,o).
GATE_POS = [(0, 0), (1, 128), (2, 384), (3, 256)]  # (pos, w_col)


def build_program(S_=S, BL_=BL):
    TB = S_ * BL_           # tokens per core
    CH = min(512, TB)       # column chunk for wide matmuls
    NCH = TB // CH
    NGT = TB // 128         # gather tiles
    PSTEPS = 8              # recurrence P-stream chunk, in time steps
    assert S_ % PSTEPS == 0 and TB % CH == 0 and TB % 128 == 0

    nc = bacc.Bacc(None, target_bir_lowering=False)

    # ---- DRAM I/O ----
    x_d = nc.dram_tensor("x", [BL_, S_], I32, kind="ExternalInput")
    tags_d = nc.dram_tensor("tags_tb", [1, TB], F32, kind="ExternalInput")
    emb_d = nc.dram_tensor("emb", [V, E], F32, kind="ExternalInput")
    wih_f_d = nc.dram_tensor("wihT_f", [E, 4 * H], F32, kind="ExternalInput")
    wih_b_d = nc.dram_tensor("wihT_b", [E, 4 * H], F32, kind="ExternalInput")
    bih_f_d = nc.dram_tensor("bihT_f", [1, 4 * H], F32, kind="ExternalInput")
    bih_b_d = nc.dram_tensor("bihT_b", [1, 4 * H], F32, kind="ExternalInput")
    whh_f_d = nc.dram_tensor("whhT_f", [H, 4 * H], BF16, kind="ExternalInput")
    whh_b_d = nc.dram_tensor("whhT_b", [H, 4 * H], BF16, kind="ExternalInput")
    wout_d = nc.dram_tensor("woutT", [H, 2 * T], BF16, kind="ExternalInput")
    bout_d = nc.dram_tensor("b_out_c", [T, 1], F32, kind="ExternalInput")
    start_d = nc.dram_tensor("start_c", [T, 1], F32, kind="ExternalInput")
    end_d = nc.dram_tensor("end_c", [T, 1], F32, kind="ExternalInput")
    trans_d = nc.dram_tensor("trans", [T, T], F32, kind="ExternalInput")
    out_d = nc.dram_tensor("out", [1, BL_], F32, kind="ExternalOutput")

    p_f_d = nc.dram_tensor("P_f", [128, S_ * 64], F32, kind="Internal")
    p_b_d = nc.dram_tensor("P_b", [128, S_ * 64], F32, kind="Internal")

    with tile.TileContext(nc) as tc:
        with tc.tile_pool(name="persist", bufs=1) as pp:
            # ---- persistent SBUF tiles ----
            xsT = pp.tile([E, TB], F32, tag="xsT")
            hf = pp.tile([128, TB], BF16, tag="hf")
            hb = pp.tile([128, TB], BF16, tag="hb")
            expE = pp.tile([T, TB], F32, tag="expE")
            oh = pp.tile([T, TB], F32, tag="oh")
            wih_f = pp.tile([E, 4 * H], F32, tag="wihf")
            wih_b = pp.tile([E, 4 * H], F32, tag="wihb")
            bih_f = pp.tile([1, 4 * H], F32, tag="bihf")
            bih_b = pp.tile([1, 4 * H], F32, tag="bihb")
            whh_f = pp.tile([H, 4 * H], BF16, tag="whhf")
            whh_b = pp.tile([H, 4 * H], BF16, tag="whhb")
            wout = pp.tile([H, 2 * T], BF16, tag="wout")
            bout = pp.tile([T, 1], F32, tag="bout")
            start_t = pp.tile([T, 1], F32, tag="start")
            end_t = pp.tile([T, 1], F32, tag="end")
            trans_t = pp.tile([T, T], F32, tag="trans")
            expT = pp.tile([T, T], F32, tag="expT")
            exp_end = pp.tile([T, 1], F32, tag="expend")
            exp_start = pp.tile([T, 1], F32, tag="expstart")
            ident = pp.tile([128, 128], F32, tag="ident")
            zeros = pp.tile([128, BL_], F32, tag="zeros")
            zeros_bf = pp.tile([128, BL_], BF16, tag="zerosbf")
            ones_t1 = pp.tile([T, 1], F32, tag="onest1")
            ones_1t = pp.tile([1, T], F32, tag="ones1t")
            logZ = pp.tile([1, BL_], F32, tag="logZ")
            denom = pp.tile([1, BL_], F32, tag="denom")
            num_acc = pp.tile([T, BL_], F32, tag="numacc")
            xT_idx = pp.tile([128, NGT], I32, tag="xtidx")

            # ---- phase 0: load params & constants ----
            for sb, d in [
                (wih_f, wih_f_d), (wih_b, wih_b_d), (whh_f, whh_f_d),
                (whh_b, whh_b_d), (wout, wout_d), (bout, bout_d),
                (start_t, start_d), (end_t, end_d), (trans_t, trans_d),
                (bih_f, bih_f_d), (bih_b, bih_b_d),
            ]:
                nc.sync.dma_start(out=sb[:], in_=d[:])
            make_identity(nc, ident[:])
            nc.vector.memset(zeros[:], 0.0)
            nc.vector.memset(zeros_bf[:], 0.0)
            nc.vector.memset(ones_t1[:], 1.0)
            nc.vector.memset(ones_1t[:], 1.0)
            nc.vector.memset(logZ[:], 0.0)
            ones_ch = pp.tile([1, CH], F32, tag="onesch")
            nc.vector.memset(ones_ch[:], 1.0)
            nc.scalar.activation(expT[:], trans_t[:], AF.Exp)
            nc.scalar.activation(exp_end[:], end_t[:], AF.Exp)
            nc.scalar.activation(exp_start[:], start_t[:], AF.Exp)

            # token indices in tb order (tb = t*BL + b): xT_idx[p, k] holds token
            # tb = k*128 + p
            th = 128 // BL_
            nc.sync.dma_start(
                out=xT_idx[:],
                in_=bass.AP(x_d, 0, [[1, th], [S_, BL_], [th, NGT]]),
            )

            # ---- phase 1: gather + transpose + input projections ----
            with (
                tc.tile_pool(name="pro_ps", bufs=2, space="PSUM") as pps,
                tc.tile_pool(name="pro_ps2", bufs=4, space="PSUM") as pps2,
                tc.tile_pool(name="pro_sb", bufs=3) as psb,
            ):
                for k in range(NGT):
                    gat = psb.tile([128, E], F32, tag="gat")
                    nc.gpsimd.indirect_dma_start(
                        out=gat[:],
                        out_offset=None,
                        in_=emb_d[:],
                        in_offset=IndirectOffsetOnAxis(ap=xT_idx[:, k : k + 1], axis=0),
                    )
                    tps = pps.tile([E, 128], F32, tag="tps", space="PSUM")
                    nc.tensor.transpose(tps[:], gat[:], ident[:])
                    nc.scalar.activation(
                        xsT[0:E, k * 128 : (k + 1) * 128], tps[:], AF.Copy
                    )

                ntch = CH // BL_  # time steps per chunk
                for dir_i, (wih, bih, p_d) in enumerate(
                    [(wih_f, bih_f, p_f_d), (wih_b, bih_b, p_b_d)]
                ):
                    for c in range(NCH):
                        # gate-interleave through SBUF (strided engine writes
                        # are free; the DMA out stays contiguous)
                        stg = psb.tile([128, ntch * 64], F32, tag="stg")
                        stg_v = stg[:].rearrange(
                            "p (t g b) -> p t g b", g=4, b=BL_
                        )
                        for pos, wcol in GATE_POS:
                            pmm = pps2.tile([128, CH], F32, tag="pmm", space="PSUM")
                            nc.tensor.matmul(
                                pmm[:],
                                lhsT=wih[:, wcol : wcol + 128],
                                rhs=xsT[:, c * CH : (c + 1) * CH],
                                start=True,
                                stop=False,
                            )
                            nc.tensor.matmul(
                                pmm[:],
                                lhsT=bih[:, wcol : wcol + 128],
                                rhs=ones_ch[:],
                                start=False,
                                stop=True,
                            )
                            nc.scalar.activation(
                                stg_v[:, :, pos, :],
                                pmm[:].rearrange("p (t b) -> p t b", b=BL_),
                                AF.Copy,
                            )
                        nc.sync.dma_start(
                            out=p_d[:, c * ntch * 64 : (c + 1) * ntch * 64],
                            in_=stg[:],
                        )

            # ---- phase 2: fwd+bwd LSTM recurrences, interleaved ----
            with (
                tc.tile_pool(name="rec_ps", bufs=4, space="PSUM") as rps,
                tc.tile_pool(name="rec_sb", bufs=4) as rsb,
                tc.tile_pool(name="pchunk", bufs=3) as pch,
            ):
                pf_chunk = pb_chunk = None
                c_prev = {0: zeros, 1: zeros}
                c_slice = {0: None, 1: None}
                for t in range(S_):
                    tb_ = S_ - 1 - t
                    if t % PSTEPS == 0:
                        pf_chunk = pch.tile([128, PSTEPS * 64], F32, tag="pfch")
                        nc.sync.dma_start(
                            out=pf_chunk[:],
                            in_=p_f_d[:, (t // PSTEPS) * PSTEPS * 64 :][
                                :, : PSTEPS * 64
                            ],
                        )
                        pb_chunk = pch.tile([128, PSTEPS * 64], F32, tag="pbch")
                        nc.sync.dma_start(
                            out=pb_chunk[:],
                            in_=p_b_d[:, (tb_ // PSTEPS) * PSTEPS * 64 :][
                                :, : PSTEPS * 64
                            ],
                        )
                    for dir_i in (0, 1):
                        if dir_i == 0:
                            tt, hstore, whh, pchk = t, hf, whh_f, pf_chunk
                            h_prev = (
                                zeros_bf[:]
                                if t == 0
                                else hf[:, (t - 1) * BL_ : t * BL_]
                            )
                        else:
                            tt, hstore, whh, pchk = tb_, hb, whh_b, pb_chunk
                            h_prev = (
                                zeros_bf[:]
                                if t == 0
                                else hb[:, (tb_ + 1) * BL_ : (tb_ + 2) * BL_]
                            )
                        psl = pchk[:, (tt % PSTEPS) * 64 : (tt % PSTEPS) * 64 + 64]
                        gps = rps.tile([128, 64], F32, tag="gps", space="PSUM")
                        for pos, wcol in GATE_POS:
                            nc.tensor.matmul(
                                gps[:, pos * BL_ : (pos + 1) * BL_],
                                lhsT=whh[:, wcol : wcol + 128],
                                rhs=h_prev,
                                start=True,
                                stop=True,
                            )
                        gates = rsb.tile([128, 64], F32, tag="gates")
                        nc.vector.tensor_tensor(
                            out=gates[:], in0=gps[:], in1=psl, op=ALU.add
                        )
                        act = rsb.tile([128, 48], F32, tag="act")
                        nc.scalar.activation(act[:], gates[:, 0:48], AF.Sigmoid)
                        tg = rsb.tile([128, BL_], F32, tag="tg")
                        nc.scalar.activation(tg[:], gates[:, 48:64], AF.Tanh)
                        c_new = rsb.tile([128, BL_], F32, tag=f"c{dir_i}")
                        # c = sig(f)*c_prev + sig(i)*tanh(g)
                        m1 = rsb.tile([128, BL_], F32, tag="m1")
                        nc.vector.tensor_tensor(
                            out=m1[:], in0=act[:, 0:BL_], in1=tg[:], op=ALU.mult
                        )
                        nc.vector.tensor_tensor(
                            out=c_new[:],
                            in0=act[:, BL_ : 2 * BL_],
                            in1=c_prev[dir_i][:] if c_slice[dir_i] is None else c_slice[dir_i],
                            op=ALU.mult,
                        )
                        nc.vector.tensor_tensor(
                            out=c_new[:], in0=c_new[:], in1=m1[:], op=ALU.add
                        )
                        tc_t = rsb.tile([128, BL_], F32, tag="tct")
                        nc.scalar.activation(tc_t[:], c_new[:], AF.Tanh)
                        nc.vector.tensor_tensor(
                            out=hstore[:, tt * BL_ : (tt + 1) * BL_],
                            in0=act[:, 2 * BL_ : 3 * BL_],
                            in1=tc_t[:],
                            op=ALU.mult,
                        )
                        c_prev[dir_i] = c_new
                        c_slice[dir_i] = c_new[:]

            # ---- phase 3: emissions + exp ----
            with (
                tc.tile_pool(name="em_ps", bufs=3, space="PSUM") as eps,
                tc.tile_pool(name="em_sb", bufs=3) as esb,
            ):
                for c in range(NCH):
                    cs = slice(c * CH, (c + 1) * CH)
                    emp = eps.tile([T, CH], F32, tag="emp", space="PSUM")
                    nc.tensor.matmul(
                        emp[:], lhsT=wout[:, 0:T], rhs=hf[:, cs],
                        start=True, stop=False,
                    )
                    nc.tensor.matmul(
                        emp[:], lhsT=wout[:, T : 2 * T], rhs=hb[:, cs],
                        start=False, stop=True,
                    )
                    em_tmp = esb.tile([T, CH], F32, tag="emtmp")
                    nc.vector.tensor_scalar_add(em_tmp[:], emp[:], bout[:])
                    nc.scalar.activation(expE[:, cs], em_tmp[:], AF.Exp)

            # ---- phase 4: CRF forward DP (exp domain), 2 half-chains ----
            # ---- phase 5 (interleaved after): numerator ----
            with (
                tc.tile_pool(name="dp_ps", bufs=2, space="PSUM") as dps,
                tc.tile_pool(name="dp_sb", bufs=3) as dsb,
                tc.tile_pool(name="nm_ps", bufs=1, space="PSUM") as nps,
                tc.tile_pool(name="nm_sb", bufs=3) as nsb,
            ):
                HB = BL_ // 2
                a_cur = [None, None]
                for hh in range(2):
                    a0 = dsb.tile([T, HB], F32, tag=f"a{hh}")
                    nc.vector.tensor_scalar_mul(
                        a0[:], expE[:, hh * HB : hh * HB + HB], exp_start[:]
                    )
                    a_cur[hh] = a0
                for t in range(1, S_):
                    for hh in range(2):
                        es = slice(t * BL_ + hh * HB, t * BL_ + hh * HB + HB)
                        aps_ = dps.tile([T, HB], F32, tag=f"aps{hh}", space="PSUM")
                        nc.tensor.matmul(
                            aps_[:], lhsT=expT[:], rhs=a_cur[hh][:],
                            start=True, stop=True,
                        )
                        a_new = dsb.tile([T, HB], F32, tag=f"a{hh}")
                        nc.vector.tensor_tensor(
                            out=a_new[:], in0=aps_[:], in1=expE[:, es], op=ALU.mult
                        )
                        a_cur[hh] = a_new
                        if t % NORM_EVERY == 0 or t == S_ - 1:
                            hs = slice(hh * HB, (hh + 1) * HB)
                            sps = dps.tile([1, HB], F32, tag="sm", space="PSUM")
                            nc.tensor.matmul(
                                sps[:], lhsT=ones_t1[:], rhs=a_new[:],
                                start=True, stop=True,
                            )
                            rec = dsb.tile([1, HB], F32, tag=f"rec{hh}")
                            nc.vector.reciprocal(rec[:], sps[:])
                            lns = dsb.tile([1, HB], F32, tag=f"lns{hh}")
                            nc.scalar.activation(lns[:], sps[:], AF.Ln)
                            nc.vector.tensor_tensor(
                                out=logZ[:, hs], in0=logZ[:, hs], in1=lns[:],
                                op=ALU.add,
                            )
                            bps = dps.tile([T, HB], F32, tag="sm", space="PSUM")
                            nc.tensor.matmul(
                                bps[:], lhsT=ones_1t[:], rhs=rec[:],
                                start=True, stop=True,
                            )
                            nc.vector.tensor_tensor(
                                out=a_new[:], in0=a_new[:], in1=bps[:], op=ALU.mult
                            )
                # final: denom = logZ + ln(sum_j A * exp(end))
                for hh in range(2):
                    hs = slice(hh * HB, (hh + 1) * HB)
                    ea = dsb.tile([T, HB], F32, tag=f"ea{hh}")
                    nc.vector.tensor_scalar_mul(ea[:], a_cur[hh][:], exp_end[:])
                    fps = dps.tile([1, HB], F32, tag="sm", space="PSUM")
                    nc.tensor.matmul(
                        fps[:], lhsT=ones_t1[:], rhs=ea[:], start=True, stop=True
                    )
                    lnf = dsb.tile([1, HB], F32, tag=f"lnf{hh}")
                    nc.scalar.activation(lnf[:], fps[:], AF.Ln)
                    nc.vector.tensor_tensor(
                        out=denom[:, hs], in0=logZ[:, hs], in1=lnf[:], op=ALU.add
                    )

                # ---- numerator ----
                iot_i = nsb.tile([T, 1], I32, tag="ioti")
                nc.gpsimd.iota(iot_i[:], pattern=[[0, 1]], base=0, channel_multiplier=1)
                iot_f = nsb.tile([T, 1], F32, tag="iotf")
                nc.vector.tensor_copy(iot_f[:], iot_i[:])
                for c in range(NCH):
                    cs = slice(c * CH, (c + 1) * CH)
                    tgc = nsb.tile([1, CH], F32, tag="tgc")
                    nc.sync.dma_start(out=tgc[:], in_=tags_d[:, cs])
                    tbps = nps.tile([T, CH], F32, tag="trp", space="PSUM")
                    nc.tensor.matmul(
                        tbps[:], lhsT=ones_1t[:], rhs=tgc[:], start=True, stop=True
                    )
                    nc.vector.tensor_tensor(
                        out=oh[:, cs], in0=tbps[:],
                        in1=iot_f[:].to_broadcast([T, CH]), op=ALU.is_equal,
                    )
                # start/end contributions
                nc.vector.tensor_scalar_mul(num_acc[:], oh[:, 0:BL_], start_t[:])
                tmp_e = nsb.tile([T, BL_], F32, tag="tmpe")
                nc.vector.tensor_scalar_mul(
                    tmp_e[:], oh[:, TB - BL_ : TB], end_t[:]
                )
                nc.vector.tensor_tensor(
                    out=num_acc[:], in0=num_acc[:], in1=tmp_e[:], op=ALU.add
                )
                ntch = CH // BL_
                for c in range(NCH):
                    cs = slice(c * CH, (c + 1) * CH)
                    # emissions along the gold path (em recomputed as ln(expE))
                    lem = nsb.tile([T, CH], F32, tag="lem")
                    nc.scalar.activation(lem[:], expE[:, cs], AF.Ln)
                    prod = nsb.tile([T, CH], F32, tag="prod")
                    nc.vector.tensor_tensor(
                        out=prod[:], in0=lem[:], in1=oh[:, cs], op=ALU.mult
                    )
                    part = nsb.tile([T, BL_], F32, tag="part")
                    nc.vector.reduce_sum(
                        part[:],
                        prod[:].rearrange("p (t b) -> p b t", b=BL_),
                        axis=AX.X,
                    )
                    nc.vector.tensor_tensor(
                        out=num_acc[:], in0=num_acc[:], in1=part[:], op=ALU.add
                    )
                    # transition scores trans[tag_t, tag_{t+1}] for t in chunk
                    trp = nps.tile([T, CH], F32, tag="trp", space="PSUM")
                    nc.tensor.matmul(
                        trp[:], lhsT=trans_t[:], rhs=oh[:, cs], start=True, stop=True
                    )
                    npair = ntch if c < NCH - 1 else ntch - 1
                    if npair <= 0:
                        continue
                    prod2 = nsb.tile([T, CH], F32, tag="prod")
                    nc.vector.tensor_tensor(
                        out=prod2[:, : npair * BL_],
                        in0=trp[:, : npair * BL_],
                        in1=oh[:, c * CH + BL_ : c * CH + BL_ + npair * BL_],
                        op=ALU.mult,
                    )
                    part2 = nsb.tile([T, BL_], F32, tag="part")
                    nc.vector.reduce_sum(
                        part2[:],
                        prod2[:, : npair * BL_].rearrange(
                            "p (t b) -> p b t", b=BL_
                        ),
                        axis=AX.X,
                    )
                    nc.vector.tensor_tensor(
                        out=num_acc[:], in0=num_acc[:], in1=part2[:], op=ALU.add
                    )
                # score per sequence, then out = score - denom
                scp = nps.tile([1, BL_], F32, tag="scp", space="PSUM")
                nc.tensor.matmul(
                    scp[:], lhsT=ones_t1[:], rhs=num_acc[:], start=True, stop=True
                )
                res = nsb.tile([1, BL_], F32, tag="res")
                nc.vector.tensor_tensor(
                    out=res[:], in0=scp[:], in1=denom[:], op=ALU.subtract
                )
                nc.sync.dma_start(out=out_d[:], in_=res[:])

    nc.compile()
    return nc


def make_in_maps(inputs, S_=S, BL_=BL, ncores=NCORES):
    """Shard full inputs into per-core in_maps (host-side layout prep only)."""
    x = np.asarray(inputs["x"], np.int32)
    tags = np.asarray(inputs["tags"], np.int32)
    emb = np.ascontiguousarray(np.asarray(inputs["emb"], np.float32))

    wih_f = np.ascontiguousarray(np.asarray(inputs["w_ih_f"], np.float32).T)
    wih_b = np.ascontiguousarray(np.asarray(inputs["w_ih_b"], np.float32).T)
    bih_f = np.ascontiguousarray(np.asarray(inputs["b_f"], np.float32)[None, :])
    bih_b = np.ascontiguousarray(np.asarray(inputs["b_b"], np.float32)[None, :])
    import ml_dtypes
    whh_f = np.ascontiguousarray(np.asarray(inputs["w_hh_f"], np.float32).T.astype(ml_dtypes.bfloat16))
    whh_b = np.ascontiguousarray(np.asarray(inputs["w_hh_b"], np.float32).T.astype(ml_dtypes.bfloat16))
    W_out = np.asarray(inputs["W_out"], np.float32)
    wout = np.ascontiguousarray(
        np.concatenate([W_out[:, :H].T, W_out[:, H:].T], 1).astype(ml_dtypes.bfloat16)
    )
    bout = np.ascontiguousarray(np.asarray(inputs["b_out"], np.float32)[:, None])
    start_c = np.ascontiguousarray(
        np.asarray(inputs["start_trans"], np.float32)[:, None]
    )
    end_c = np.ascontiguousarray(np.asarray(inputs["end_trans"], np.float32)[:, None])
    trans = np.ascontiguousarray(np.asarray(inputs["trans"], np.float32))

    in_maps = []
    for c in range(ncores):
        xs = np.ascontiguousarray(x[c * BL_ : (c + 1) * BL_])
        tg = tags[c * BL_ : (c + 1) * BL_]
        tags_tb = np.ascontiguousarray(
            tg.T.reshape(1, -1).astype(np.float32)
        )  # t-major [1, S*BL]
        in_maps.append(
            {
                "x": xs,
                "tags_tb": tags_tb,
                "emb": emb,
                "wihT_f": wih_f,
                "wihT_b": wih_b,
                "bihT_f": bih_f,
                "bihT_b": bih_b,
                "whhT_f": whh_f,
                "whhT_b": whh_b,
                "woutT": wout,
                "b_out_c": bout,
                "start_c": start_c,
                "end_c": end_c,
                "trans": trans,
            }
        )
    return in_maps


_NC_CACHE = {}


def _install_ntff_hook_shim():
    """The agent image's antenv lacks axon_hooks; replicate the ctypes NTFF
    profile hook (see trn_agent_boot/trn_boot.py) so trace=True works."""
    import contextlib
    import ctypes
    import types

    if "antenv.axon_hooks" in sys.modules:
        return
    so_path = "/opt/axon/libaxon_pjrt.so"
    try:
        lib = ctypes.CDLL(so_path)
    except OSError:
        return
    if not hasattr(lib, "axon_start_nrt_profile"):
        return
    lib.axon_start_nrt_profile.argtypes = [
        ctypes.POINTER(ctypes.c_int64),
        ctypes.c_size_t,
    ]
    lib.axon_start_nrt_profile.restype = ctypes.c_int64
    lib.axon_stop_nrt_profile.argtypes = [ctypes.c_char_p]
    lib.axon_stop_nrt_profile.restype = ctypes.c_int64

    @contextlib.contextmanager
    def _hook(output_dir, device_ids):
        import jax

        jax.devices()
        if device_ids:
            ids = (ctypes.c_int64 * len(device_ids))(*device_ids)
            rc = lib.axon_start_nrt_profile(ids, len(device_ids))
        else:
            rc = lib.axon_start_nrt_profile(None, 0)
        if rc != 0:
            raise RuntimeError(f"axon_start_nrt_profile rc={rc}")
        try:
            yield
        finally:
            n = lib.axon_stop_nrt_profile(str(output_dir).encode())
            print(f"profile: {n} file(s) written to {output_dir}")

    mod = types.ModuleType("antenv.axon_hooks")
    mod.get_axon_ntff_profile_hook = lambda: _hook
    mod.set_axon_ntff_profile_hook = lambda h: None
    sys.modules["antenv.axon_hooks"] = mod


def kernel(**inputs):
    from concourse.bass_utils import run_bass_kernel_spmd

    if "nc" not in _NC_CACHE:
        _NC_CACHE["nc"] = build_program()
    nc = _NC_CACHE["nc"]
    in_maps = make_in_maps(inputs)
    trace = bool(int(os.environ.get("BASS_KERNEL_TRACE", "0")))
    if trace:
        _install_ntff_hook_shim()
        import concourse.bass_utils as _bu

        _orig_upload = _bu.upload_artifacts

        def _safe_upload(tmpdir):
            try:
                return _orig_upload(tmpdir)
            except Exception as e:
                print(f"upload_artifacts failed ({e}); using local dir")
                return tmpdir

        _bu.upload_artifacts = _safe_upload
    res = run_bass_kernel_spmd(
        nc, in_maps, core_ids=list(range(NCORES)), trace=trace
    )
    if trace and res.exec_time_ns is not None:
        print(f"HW exec time: {res.exec_time_ns} ns")
    parts = np.concatenate([r["out"].reshape(-1) for r in res.results])
    return np.float32(-np.mean(parts))



# revision 12
# speedup vs baseline: 1.4459x; 1.4459x over previous
"""BiLSTM-CRF loss kernel for Trainium2, data-parallel over batch on 8 NeuronCores.

Per-core program (B_local=16 sequences, S=512, T=20 tags, E=100, H=128):
  Main loop: 512-step fwd+bwd LSTM recurrence (two independent chains) with all
  producer work streamed in as background items between steps:
    - embedding gather (indirect DMA, bf16) + PE transpose -> xsT [101, S*16]
      (ones row folds the input-projection bias into the matmul),
    - input projections P = W_ih_aug @ xsT per (dir, gate, 32-step chunk),
      kept in SBUF bf16 ring buffers (no DRAM round trip),
    - one-hot of tags (for the CRF numerator).
  Per step per dir: 4x (identity-matmul P-add + W_hh matmul) accumulate gates in
  PSUM; sigmoid/tanh read PSUM; cell update split across Vector/Pool engines.
  Tail: emissions em^T = W_out @ [hf;hb] (+bias folded into Exp via per-partition
  bias), CRF numerator from PSUM pre-activations, and the CRF partition function
  as two chains meeting in the middle: alpha (t=0..255) and gamma_t = E_t * beta_t
  (t=511..256), both renormalized by the compile-time constant 2^-69 every 16
  steps (no data-dependent renorm on the critical path); the log2 bookkeeping is
  added back as a constant at the end.

mask is all ones for this problem (spec fill=ones), so masking is elided and
seq_ends = S-1.
"""

import math
import os
import sys

import numpy as np

sys.path.insert(0, "/opt/trn_rl_repo")

import concourse.bass as bass
import concourse.mybir as mybir
import concourse.tile as tile
from concourse import bacc
from concourse.bass import IndirectOffsetOnAxis
from concourse.masks import make_identity

AF = mybir.ActivationFunctionType
ALU = mybir.AluOpType
AX = mybir.AxisListType
F32 = mybir.dt.float32
BF16 = mybir.dt.bfloat16
I32 = mybir.dt.int32

V, T, E, HD = 32000, 20, 100, 256
H = 128
B, S = 128, 512
NCORES = 8
BL = B // NCORES          # 16 sequences per core
TB = S * BL               # 8192 tokens per core
CHS = 32                  # time steps per projection/emission chunk
NPC = S // CHS            # 16 chunks
NGT = TB // 128           # 64 gather tiles
RENORM = 16               # DP renorm period (steps)
RSH = 69                  # A *= 2^-69 each renorm (~20^16)
DPH = S // 2              # alpha/gamma half length


def build_program():
    nc = bacc.Bacc(None, target_bir_lowering=False)

    # ---- DRAM I/O ----
    x_d = nc.dram_tensor("x", [BL, S], I32, kind="ExternalInput")
    tags_d = nc.dram_tensor("tags_tb", [1, TB], BF16, kind="ExternalInput")
    CW = CHS * BL
    emb_d = nc.dram_tensor("emb_bf", [V, E], BF16, kind="ExternalInput")
    wih_f_d = nc.dram_tensor("wihT_f", [E + 1, 4 * H], BF16, kind="ExternalInput")
    wih_b_d = nc.dram_tensor("wihT_b", [E + 1, 4 * H], BF16, kind="ExternalInput")
    whh_f_d = nc.dram_tensor("whhT_f", [H, 4 * H], BF16, kind="ExternalInput")
    whh_b_d = nc.dram_tensor("whhT_b", [H, 4 * H], BF16, kind="ExternalInput")
    wout_d = nc.dram_tensor("woutT", [H, 2 * T], BF16, kind="ExternalInput")
    bout_d = nc.dram_tensor("b_out_c", [T, 1], F32, kind="ExternalInput")
    start_d = nc.dram_tensor("start_c", [T, 1], F32, kind="ExternalInput")
    end_d = nc.dram_tensor("end_c", [T, 1], F32, kind="ExternalInput")
    trans_d = nc.dram_tensor("trans", [T, T], F32, kind="ExternalInput")
    transT_d = nc.dram_tensor("transT", [T, T], F32, kind="ExternalInput")
    trans_bf_d = nc.dram_tensor("trans_bf", [T, T], BF16, kind="ExternalInput")
    out_d = nc.dram_tensor("out", [1, BL], F32, kind="ExternalOutput")
    DBG = bool(int(os.environ.get("BASS_KERNEL_DEBUG", "0")))
    if DBG:
        dbg_hf = nc.dram_tensor("dbg_hf", [128, 4 * BL], F32, kind="ExternalOutput")
        dbg_hb = nc.dram_tensor("dbg_hb", [128, 4 * BL], F32, kind="ExternalOutput")
        dbg_ee = nc.dram_tensor("dbg_ee", [T, 4 * BL], F32, kind="ExternalOutput")
        dbg_na = nc.dram_tensor("dbg_na", [T, BL], F32, kind="ExternalOutput")
        dbg_al = nc.dram_tensor("dbg_al", [T, BL], F32, kind="ExternalOutput")
        dbg_ga = nc.dram_tensor("dbg_ga", [T, BL], F32, kind="ExternalOutput")
        dbg_oh = nc.dram_tensor("dbg_oh", [T, 4 * BL], F32, kind="ExternalOutput")
        dbg_fin = nc.dram_tensor("dbg_fin", [T, BL], F32, kind="ExternalOutput")
        dbg_lnf = nc.dram_tensor("dbg_lnf", [1, BL], F32, kind="ExternalOutput")
        dbg_scp = nc.dram_tensor("dbg_scp", [1, BL], F32, kind="ExternalOutput")

    with tile.TileContext(nc) as tc:
        with tc.tile_pool(name="persist", bufs=1) as pp:
            # ---- persistent SBUF tiles ----
            xsT = pp.tile([E + 1, TB], BF16, tag="xsT")
            hf = pp.tile([128, TB], BF16, tag="hf")
            hb = pp.tile([128, TB], BF16, tag="hb")
            expE = pp.tile([T, TB], F32, tag="expE")
            esc = pp.tile([T, 2 * CHS * BL], F32, tag="esc")  # 2^-69-scaled slices
            oh = pp.tile([T, TB], BF16, tag="oh")
            wih_f = pp.tile([E + 1, 4 * H], BF16, tag="wihf")
            wih_b = pp.tile([E + 1, 4 * H], BF16, tag="wihb")
            whh_f = pp.tile([H, 4 * H], BF16, tag="whhf")
            whh_b = pp.tile([H, 4 * H], BF16, tag="whhb")
            wout = pp.tile([H, 2 * T], BF16, tag="wout")
            bout = pp.tile([T, 1], F32, tag="bout")
            start_t = pp.tile([T, 1], F32, tag="start")
            end_t = pp.tile([T, 1], F32, tag="end")
            trans_bf = pp.tile([T, T], BF16, tag="transbf")
            expT = pp.tile([T, T], F32, tag="expT")
            expTT = pp.tile([T, T], F32, tag="expTT")
            exp_end = pp.tile([T, 1], F32, tag="expend")
            exp_start = pp.tile([T, 1], F32, tag="expstart")
            identb = pp.tile([128, 128], BF16, tag="identb")
            ones_t1 = pp.tile([T, 1], F32, tag="onest1")
            rs_t1 = pp.tile([T, 1], F32, tag="rst1")  # 2^-69 column for the
            # final colsum so Ln's input lands in a sane range
            ones_1t = pp.tile([1, T], BF16, tag="ones1t")
            iot_f = pp.tile([T, 1], F32, tag="iotf")
            num_acc = pp.tile([T, BL], F32, tag="numacc")
            xT_idx = pp.tile([128, NGT], I32, tag="xtidx")

            # ---- param loads & constants ----
            tmp_tr = pp.tile([T, T], F32, tag="tmptr")
            for sb, d in [
                (wih_f, wih_f_d), (wih_b, wih_b_d), (whh_f, whh_f_d),
                (whh_b, whh_b_d), (wout, wout_d), (bout, bout_d),
                (start_t, start_d), (end_t, end_d), (trans_bf, trans_bf_d),
            ]:
                nc.sync.dma_start(out=sb[:], in_=d[:])
            nc.sync.dma_start(out=tmp_tr[:], in_=trans_d[:])
            nc.scalar.activation(expT[:], tmp_tr[:], AF.Exp)
            tmp_tr2 = pp.tile([T, T], F32, tag="tmptr2")
            nc.sync.dma_start(out=tmp_tr2[:], in_=transT_d[:])
            nc.scalar.activation(expTT[:], tmp_tr2[:], AF.Exp)
            nc.scalar.activation(exp_end[:], end_t[:], AF.Exp)
            nc.scalar.activation(exp_start[:], start_t[:], AF.Exp)
            make_identity(nc, identb[:])
            nc.vector.memset(ones_t1[:], 1.0)
            nc.vector.memset(rs_t1[:], float(2.0 ** (-RSH)))
            nc.vector.memset(ones_1t[:], 1.0)
            # ones row for the bias: engines need 32-aligned partition bases,
            # so memset partitions 96..100 and let the transposes overwrite
            # 96..99 with real embedding data afterwards.
            nc.vector.memset(xsT[96 : E + 1, :], 1.0)
            iot_i = pp.tile([T, 1], I32, tag="ioti")
            nc.gpsimd.iota(iot_i[:], pattern=[[0, 1]], base=0, channel_multiplier=1)
            nc.vector.tensor_copy(iot_f[:], iot_i[:])

            # token indices in tb order (tb = t*BL + b): xT_idx[p, k] = k*128 + p
            th = 128 // BL
            nc.sync.dma_start(
                out=xT_idx[:],
                in_=bass.AP(x_d, 0, [[1, th], [S, BL], [th, NGT]]),
            )

            with (
                tc.tile_pool(name="gat_sb", bufs=3) as gsb,
                tc.tile_pool(name="wide_ps", bufs=2, space="PSUM") as wps,
                tc.tile_pool(name="g_ps", bufs=3, space="PSUM") as gps_pool,
                tc.tile_pool(name="p_sb", bufs=2) as psb,
                tc.tile_pool(name="cell_sb", bufs=3) as csb,
            ):
                # ---------- background item emitters ----------
                p_tiles = {}

                def emit_gather(k):
                    gat = gsb.tile([128, E], BF16, tag="gat", name="gat")
                    nc.gpsimd.indirect_dma_start(
                        out=gat[:],
                        out_offset=None,
                        in_=emb_d[:],
                        in_offset=IndirectOffsetOnAxis(ap=xT_idx[:, k : k + 1], axis=0),
                    )
                    return gat

                def emit_transpose(k, gat):
                    wtile = wps.tile([128, 1024], BF16, tag="wide", name="wtile")
                    tps = wtile[0:E, 0:128]
                    nc.tensor.transpose(tps, gat[:], identb[:])
                    nc.vector.tensor_copy(xsT[0:E, k * 128 : (k + 1) * 128], tps)

                def emit_proj(dir_i, ci, g):
                    # one gate of one 32-step chunk: P[g] = wih_aug[:, g].T @ xsT
                    wih = wih_f if dir_i == 0 else wih_b
                    wtile = wps.tile([128, 1024], BF16, tag="wide", name="wtile")
                    pmm = wtile[:].bitcast(F32)
                    nc.tensor.matmul(
                        pmm,
                        lhsT=wih[:, g * 128 : (g + 1) * 128],
                        rhs=xsT[:, ci * CHS * BL : (ci + 1) * CHS * BL],
                        start=True, stop=True,
                    )
                    pt = psb.tile([128, CHS * BL], BF16, tag=f"p{dir_i}{g}",
                                  name="pt")
                    nc.vector.tensor_copy(pt[:], pmm)
                    p_tiles[(dir_i, ci, g)] = pt

                def emit_oh(c):
                    # one-hot of tags for chunk c (tags only; no recurrence dep)
                    cs = slice(c * CHS * BL, (c + 1) * CHS * BL)
                    tgc = gsb.tile([1, CW], BF16, tag="tgc", name="tgc")
                    nc.sync.dma_start(out=tgc[:], in_=tags_d[:, cs])
                    wtile = wps.tile([128, 1024], BF16, tag="wide", name="wtile")
                    ohp = wtile[0:T, :].bitcast(F32)
                    nc.tensor.matmul(
                        ohp, lhsT=ones_1t[:], rhs=tgc[:],
                        start=True, stop=True,
                    )
                    nc.vector.tensor_tensor(
                        out=oh[:, cs], in0=ohp,
                        in1=iot_f[:].to_broadcast([T, CHS * BL]), op=ALU.is_equal,
                    )

                # ---------- background schedule ----------
                def tiles_for(ci):
                    return list(range(4 * ci, 4 * ci + 4))

                prologue = []
                for k in tiles_for(0) + tiles_for(15):
                    prologue.append(("gath", k))
                for dir_i, ci in [(0, 0), (1, 15)]:
                    for g in range(4):
                        prologue.append(("proj", dir_i, ci, g))
                windows = {i: [] for i in range(1, 16)}
                for i in range(1, 8):
                    for k in tiles_for(i) + tiles_for(15 - i):
                        windows[i].append(("gath", k))
                for i in range(1, 16):
                    for g in range(4):
                        windows[i].append(("proj", 0, i, g))
                    for g in range(4):
                        windows[i].append(("proj", 1, 15 - i, g))
                for c in range(NPC):
                    windows[(c % 15) + 1].append(("oh", c))

                gat_tiles = {}

                def run_item(item):
                    if item[0] == "gath":
                        gat_tiles[item[1]] = emit_gather(item[1])
                        # transpose immediately after (PE + DVE, cheap)
                        emit_transpose(item[1], gat_tiles[item[1]])
                    elif item[0] == "proj":
                        emit_proj(item[1], item[2], item[3])
                    else:
                        emit_oh(item[1])

                for item in prologue:
                    run_item(item)

                # ---------- main recurrence ----------
                c_slice = {0: None, 1: None}
                wq, wlen, qi = [], 0, 0
                for t in range(S):
                    if t % CHS == 0:
                        wq = windows.get(t // CHS + 1, [])
                        wlen, qi = len(wq), 0
                    # spread this window's items over its 32 steps
                    target = ((t % CHS) + 1) * wlen // CHS
                    while qi < target:
                        run_item(wq[qi])
                        qi += 1
                    tb_ = S - 1 - t
                    for dir_i in (0, 1):
                        if dir_i == 0:
                            tt, hstore, whh = t, hf, whh_f
                            h_prev = (
                                None if t == 0
                                else hf[:, (t - 1) * BL : t * BL]
                            )
                        else:
                            tt, hstore, whh = tb_, hb, whh_b
                            h_prev = (
                                None if t == 0
                                else hb[:, (tb_ + 1) * BL : (tb_ + 2) * BL]
                            )
                        ci = tt // CHS
                        to = tt % CHS
                        g_ps = gps_pool.tile([128, 64], F32, tag=f"g{dir_i}",
                                             name="g_ps", space="PSUM")
                        for g in range(4):
                            pslice = p_tiles[(dir_i, ci, g)][:, to * BL : (to + 1) * BL]
                            if t == 0:
                                nc.tensor.matmul(
                                    g_ps[:, g * BL : (g + 1) * BL],
                                    lhsT=identb[:], rhs=pslice,
                                    start=True, stop=True,
                                )
                            else:
                                nc.tensor.matmul(
                                    g_ps[:, g * BL : (g + 1) * BL],
                                    lhsT=identb[:], rhs=pslice,
                                    start=True, stop=False,
                                )
                                nc.tensor.matmul(
                                    g_ps[:, g * BL : (g + 1) * BL],
                                    lhsT=whh[:, g * 128 : (g + 1) * 128],
                                    rhs=h_prev,
                                    start=False, stop=True,
                                )
                        # gate cols: [i f o | g]
                        sig = csb.tile([128, 48], F32, tag=f"sig{dir_i}", name="sig")
                        nc.scalar.activation(sig[:], g_ps[:, 0:48], AF.Sigmoid)
                        tg = csb.tile([128, BL], F32, tag=f"tg{dir_i}", name="tg")
                        nc.scalar.activation(tg[:], g_ps[:, 48:64], AF.Tanh)
                        c_new = csb.tile([128, BL], F32, tag=f"c{dir_i}", name="c_new")
                        if t == 0:
                            # c = i*g
                            nc.vector.tensor_tensor(
                                out=c_new[:], in0=sig[:, 0:BL], in1=tg[:],
                                op=ALU.mult,
                            )
                        else:
                            m1 = csb.tile([128, BL], F32, tag=f"m1{dir_i}", name="m1")
                            nc.vector.tensor_tensor(
                                out=m1[:], in0=sig[:, 0:BL], in1=tg[:], op=ALU.mult
                            )
                            fc = csb.tile([128, BL], F32, tag=f"fc{dir_i}", name="fc")
                            nc.gpsimd.tensor_tensor(
                                out=fc[:], in0=sig[:, BL : 2 * BL],
                                in1=c_slice[dir_i], op=ALU.mult,
                            )
                            if dir_i == 0:
                                nc.vector.tensor_tensor(
                                    out=c_new[:], in0=m1[:], in1=fc[:], op=ALU.add
                                )
                            else:
                                nc.gpsimd.tensor_tensor(
                                    out=c_new[:], in0=m1[:], in1=fc[:], op=ALU.add
                                )
                        tc_t = csb.tile([128, BL], F32, tag=f"tct{dir_i}", name="tc_t")
                        nc.scalar.activation(tc_t[:], c_new[:], AF.Tanh)
                        nc.vector.tensor_tensor(
                            out=hstore[:, tt * BL : (tt + 1) * BL],
                            in0=sig[:, 2 * BL : 3 * BL], in1=tc_t[:], op=ALU.mult,
                        )
                        c_slice[dir_i] = c_new[:]

            # ---------- emissions + numerator + CRF DP ----------
            RS = float(2.0 ** (-RSH))
            with (
                tc.tile_pool(name="em_ps", bufs=2, space="PSUM") as eps,  # tag "ew" shared: 2 banks
                tc.tile_pool(name="em_sb", bufs=3) as esb,
                tc.tile_pool(name="dp_ps", bufs=4, space="PSUM") as dps,  # tag "dp" shared: 4 banks
                tc.tile_pool(name="dp_sb", bufs=3) as dsb,
            ):
                # start/end contributions to the numerator need oh (built above)
                nc.vector.tensor_scalar_mul(num_acc[:], oh[:, 0:BL], start_t[:])
                tmp_e = esb.tile([T, BL], F32, tag="tmpe")
                nc.vector.tensor_scalar_mul(tmp_e[:], oh[:, TB - BL : TB], end_t[:])
                nc.vector.tensor_tensor(
                    out=num_acc[:], in0=num_acc[:], in1=tmp_e[:], op=ALU.add
                )

                a_cur = None
                g_cur = None
                na = 0
                ng = 0

                def em_chunk(c):
                    CW = CHS * BL
                    cs = slice(c * CW, (c + 1) * CW)
                    emp = eps.tile([T, CW], F32, tag="ew", name="emp", space="PSUM")
                    nc.tensor.matmul(
                        emp[:], lhsT=wout[:, 0:T], rhs=hf[:, cs],
                        start=True, stop=False,
                    )
                    nc.tensor.matmul(
                        emp[:], lhsT=wout[:, T : 2 * T], rhs=hb[:, cs],
                        start=False, stop=True,
                    )
                    # expE = exp(em + b_out)  (bias folded into the activation)
                    nc.scalar.activation(expE[:, cs], emp[:], AF.Exp, bias=bout[:])
                    # pre-scaled slices for the DP renorm
                    for s in range(c * CHS, (c + 1) * CHS):
                        if s % RENORM == 0 and s >= RENORM:
                            col = (s // RENORM) * BL
                            nc.vector.tensor_scalar_mul(
                                esc[:, col : col + BL],
                                expE[:, s * BL : (s + 1) * BL],
                                RS,
                            )
                    # numerator: emissions along the gold path (from PSUM pre-act)
                    prod = esb.tile([T, CW], F32, tag="prod", name="prod")
                    nc.vector.scalar_tensor_tensor(
                        out=prod[:], in0=emp[:], scalar=bout[:], in1=oh[:, cs],
                        op0=ALU.add, op1=ALU.mult,
                    )
                    part = esb.tile([T, BL], F32, tag="part", name="part")
                    nc.vector.reduce_sum(
                        part[:], prod[:].rearrange("p (t b) -> p b t", b=BL),
                        axis=AX.X,
                    )
                    nc.gpsimd.tensor_tensor(
                        out=num_acc[:], in0=num_acc[:], in1=part[:], op=ALU.add
                    )
                    # transition scores trans[tag_t, tag_{t+1}]
                    trp = eps.tile([T, CW], F32, tag="ew", name="trp", space="PSUM")
                    nc.tensor.matmul(
                        trp[:], lhsT=trans_bf[:], rhs=oh[:, cs],
                        start=True, stop=True,
                    )
                    npair = CHS if c < NPC - 1 else CHS - 1
                    prod2 = esb.tile([T, CW], F32, tag="prod", name="prod2")
                    nc.vector.tensor_tensor(
                        out=prod2[:, : npair * BL],
                        in0=trp[:, : npair * BL],
                        in1=oh[:, c * CW + BL : c * CW + BL + npair * BL],
                        op=ALU.mult,
                    )
                    part2 = esb.tile([T, BL], F32, tag="part", name="part2")
                    nc.vector.reduce_sum(
                        part2[:],
                        prod2[:, : npair * BL].rearrange("p (t b) -> p b t", b=BL),
                        axis=AX.X,
                    )
                    nc.gpsimd.tensor_tensor(
                        out=num_acc[:], in0=num_acc[:], in1=part2[:], op=ALU.add
                    )

                def alpha_steps(lo, hi):
                    nonlocal a_cur, na
                    for s in range(lo, hi):
                        if s == 0:
                            a0 = dsb.tile([T, BL], F32, tag="al", name="a0")
                            nc.vector.tensor_scalar_mul(
                                a0[:], expE[:, 0:BL], exp_start[:]
                            )
                            a_cur = a0
                            continue
                        aps = dps.tile([T, BL], F32, tag="dp", name="aps",
                                       space="PSUM")
                        nc.tensor.matmul(
                            aps[:], lhsT=expT[:], rhs=a_cur[:],
                            start=True, stop=True,
                        )
                        if s % RENORM == 0:
                            e_sl = esc[:, (s // RENORM) * BL :][:, :BL]
                            na += 1
                        else:
                            e_sl = expE[:, s * BL : (s + 1) * BL]
                        a_new = dsb.tile([T, BL], F32, tag="al", name="a_new")
                        nc.vector.tensor_tensor(
                            out=a_new[:], in0=aps[:], in1=e_sl, op=ALU.mult
                        )
                        a_cur = a_new

                def gamma_steps(hi, lo):
                    # processes s = hi-1 ... lo (gamma_s = E_s * (M gamma_{s+1}))
                    nonlocal g_cur, ng
                    for s in range(hi - 1, lo - 1, -1):
                        if s == S - 1:
                            g0 = dsb.tile([T, BL], F32, tag="ga", name="g0")
                            nc.vector.tensor_scalar_mul(
                                g0[:], expE[:, (S - 1) * BL :][:, :BL], exp_end[:]
                            )
                            g_cur = g0
                            continue
                        gp = dps.tile([T, BL], F32, tag="dp", name="gp",
                                      space="PSUM")
                        nc.tensor.matmul(
                            gp[:], lhsT=expTT[:], rhs=g_cur[:],
                            start=True, stop=True,
                        )
                        if s % RENORM == 0:
                            e_sl = esc[:, (s // RENORM) * BL :][:, :BL]
                            ng += 1
                        else:
                            e_sl = expE[:, s * BL : (s + 1) * BL]
                        g_new = dsb.tile([T, BL], F32, tag="ga", name="g_new")
                        nc.vector.tensor_tensor(
                            out=g_new[:], in0=gp[:], in1=e_sl, op=ALU.mult
                        )
                        g_cur = g_new

                for c in range(8):
                    em_chunk(c)
                    em_chunk(15 - c)
                    alpha_steps(c * CHS, (c + 1) * CHS)
                    gamma_steps(S - c * CHS, S - (c + 1) * CHS)

                # combine: denom = ln(sum_i gamma_256[i] * (M^T alpha_255)[i]) + C
                fps = dps.tile([T, BL], F32, tag="dp", name="fps", space="PSUM")
                nc.tensor.matmul(
                    fps[:], lhsT=expT[:], rhs=a_cur[:], start=True, stop=True
                )
                fin = dsb.tile([T, BL], F32, tag="fin", name="fin")
                nc.vector.tensor_tensor(
                    out=fin[:], in0=fps[:], in1=g_cur[:], op=ALU.mult
                )
                sps = dps.tile([1, BL], F32, tag="dp", name="sps", space="PSUM")
                nc.tensor.matmul(
                    sps[:], lhsT=rs_t1[:], rhs=fin[:], start=True, stop=True
                )
                lnf = dsb.tile([1, BL], F32, tag="lnf", name="lnf")
                nc.scalar.activation(lnf[:], sps[:], AF.Ln)
                # score per sequence
                scp = dps.tile([1, BL], F32, tag="dp", name="scp", space="PSUM")
                nc.tensor.matmul(
                    scp[:], lhsT=ones_t1[:], rhs=num_acc[:], start=True, stop=True
                )
                C = (na + ng + 1) * RSH * math.log(2.0)
                res = dsb.tile([1, BL], F32, tag="res", name="res")
                nc.vector.scalar_tensor_tensor(
                    out=res[:], in0=scp[:], scalar=-C, in1=lnf[:],
                    op0=ALU.add, op1=ALU.subtract,
                )
                nc.sync.dma_start(out=out_d[:], in_=res[:])
                if DBG:
                    nc.sync.dma_start(out=dbg_fin[:], in_=fin[:])
                    nc.sync.dma_start(out=dbg_lnf[:], in_=lnf[:])
                    scpc = dsb.tile([1, BL], F32, tag="scpc", name="scpc")
                    nc.vector.tensor_copy(scpc[:], scp[:])
                    nc.sync.dma_start(out=dbg_scp[:], in_=scpc[:])
                    dtile = dsb.tile([128, 4 * BL], F32, tag="dbg", name="dtile")
                    nc.vector.tensor_copy(dtile[:], hf[:, 0 : 4 * BL])
                    nc.sync.dma_start(out=dbg_hf[:], in_=dtile[:])
                    dtile2 = dsb.tile([128, 4 * BL], F32, tag="dbg", name="dtile2")
                    nc.vector.tensor_copy(dtile2[:], hb[:, 0 : 4 * BL])
                    nc.sync.dma_start(out=dbg_hb[:], in_=dtile2[:])
                    nc.sync.dma_start(out=dbg_ee[:], in_=expE[:, 0 : 4 * BL])
                    nc.sync.dma_start(out=dbg_na[:], in_=num_acc[:])
                    nc.sync.dma_start(out=dbg_al[:], in_=a_cur[:])
                    nc.sync.dma_start(out=dbg_ga[:], in_=g_cur[:])
                    dtile3 = dsb.tile([T, 4 * BL], F32, tag="dbg2", name="dtile3")
                    nc.vector.tensor_copy(dtile3[:], oh[:, 0 : 4 * BL])
                    nc.sync.dma_start(out=dbg_oh[:], in_=dtile3[:])

    nc.compile()
    return nc


def make_in_maps(inputs, ncores=NCORES):
    """Shard full inputs into per-core in_maps (host-side layout prep only)."""
    import ml_dtypes

    BF = ml_dtypes.bfloat16
    x = np.asarray(inputs["x"], np.int32)
    tags = np.asarray(inputs["tags"], np.int32)
    emb = np.asarray(inputs["emb"], np.float32).astype(BF)

    def reorder(w):
        # PyTorch gate order (i, f, g, o) -> kernel order (i, f, o, g)
        wi, wf, wg, wo = np.split(np.asarray(w, np.float32), 4, axis=0)
        return np.concatenate([wi, wf, wo, wg], 0)

    def aug(w_ih, b):
        w = reorder(w_ih)          # [4H, E]
        bb = reorder(np.asarray(b, np.float32)[:, None])  # [4H, 1]
        return np.ascontiguousarray(
            np.concatenate([w.T, bb.T], 0).astype(BF)
        )  # [E+1, 4H]

    wih_f = aug(inputs["w_ih_f"], inputs["b_f"])
    wih_b = aug(inputs["w_ih_b"], inputs["b_b"])
    whh_f = np.ascontiguousarray(reorder(inputs["w_hh_f"]).T.astype(BF))
    whh_b = np.ascontiguousarray(reorder(inputs["w_hh_b"]).T.astype(BF))
    W_out = np.asarray(inputs["W_out"], np.float32)
    wout = np.ascontiguousarray(
        np.concatenate([W_out[:, :H].T, W_out[:, H:].T], 1).astype(BF)
    )
    bout = np.ascontiguousarray(np.asarray(inputs["b_out"], np.float32)[:, None])
    start_c = np.ascontiguousarray(
        np.asarray(inputs["start_trans"], np.float32)[:, None]
    )
    end_c = np.ascontiguousarray(np.asarray(inputs["end_trans"], np.float32)[:, None])
    trans = np.ascontiguousarray(np.asarray(inputs["trans"], np.float32))
    transT = np.ascontiguousarray(trans.T)
    trans_bf = np.ascontiguousarray(trans.astype(BF))

    in_maps = []
    for c in range(ncores):
        xs = np.ascontiguousarray(x[c * BL : (c + 1) * BL])
        tg = tags[c * BL : (c + 1) * BL]
        tags_tb = np.ascontiguousarray(
            tg.T.reshape(1, -1).astype(np.float32).astype(BF)
        )  # t-major [1, S*BL]
        in_maps.append(
            {
                "x": xs,
                "tags_tb": tags_tb,
                "emb_bf": emb,
                "wihT_f": wih_f,
                "wihT_b": wih_b,
                "whhT_f": whh_f,
                "whhT_b": whh_b,
                "woutT": wout,
                "b_out_c": bout,
                "start_c": start_c,
                "end_c": end_c,
                "trans": trans,
                "transT": transT,
                "trans_bf": trans_bf,
            }
        )
    return in_maps


_NC_CACHE = {}


def _install_ntff_hook_shim():
    """The agent image's antenv lacks axon_hooks; replicate the ctypes NTFF
    profile hook (see trn_agent_boot/trn_boot.py) so trace=True works."""
    import contextlib
    import ctypes
    import types

    if "antenv.axon_hooks" in sys.modules:
        return
    so_path = "/opt/axon/libaxon_pjrt.so"
    try:
        lib = ctypes.CDLL(so_path)
    except OSError:
        return
    if not hasattr(lib, "axon_start_nrt_profile"):
        return
    lib.axon_start_nrt_profile.argtypes = [
        ctypes.POINTER(ctypes.c_int64),
        ctypes.c_size_t,
    ]
    lib.axon_start_nrt_profile.restype = ctypes.c_int64
    lib.axon_stop_nrt_profile.argtypes = [ctypes.c_char_p]
    lib.axon_stop_nrt_profile.restype = ctypes.c_int64

    @contextlib.contextmanager
    def _hook(output_dir, device_ids):
        import jax

        jax.devices()
        if device_ids:
            ids = (ctypes.c_int64 * len(device_ids))(*device_ids)
            rc = lib.axon_start_nrt_profile(ids, len(device_ids))
        else:
            rc = lib.axon_start_nrt_profile(None, 0)
        if rc != 0:
            raise RuntimeError(f"axon_start_nrt_profile rc={rc}")
        try:
            yield
        finally:
            n = lib.axon_stop_nrt_profile(str(output_dir).encode())
            print(f"profile: {n} file(s) written to {output_dir}")

    mod = types.ModuleType("antenv.axon_hooks")
    mod.get_axon_ntff_profile_hook = lambda: _hook
    mod.set_axon_ntff_profile_hook = lambda h: None
    sys.modules["antenv.axon_hooks"] = mod


def kernel(**inputs):
    from concourse.bass_utils import run_bass_kernel_spmd

    if "nc" not in _NC_CACHE:
        _NC_CACHE["nc"] = build_program()
    nc = _NC_CACHE["nc"]
    in_maps = make_in_maps(inputs)
    trace = bool(int(os.environ.get("BASS_KERNEL_TRACE", "0")))
    if trace:
        _install_ntff_hook_shim()
        import concourse.bass_utils as _bu

        _orig_upload = _bu.upload_artifacts

        def _safe_upload(tmpdir):
            try:
                return _orig_upload(tmpdir)
            except Exception as e:
                print(f"upload_artifacts failed ({e}); using local dir")
                return tmpdir

        _bu.upload_artifacts = _safe_upload
    res = run_bass_kernel_spmd(
        nc, in_maps, core_ids=list(range(NCORES)), trace=trace
    )
    if trace and res.exec_time_ns is not None:
        print(f"HW exec time: {res.exec_time_ns} ns")
    parts = np.concatenate([r["out"].reshape(-1) for r in res.results])
    return np.float32(-np.mean(parts))
